# revision 9
# baseline (speedup 1.0000x reference)
"""4D multilinear interpolation (8x8x8x8 lattice) on 8 Trainium2 cores.

For each row b: scale coordinates[b] (4 values in [0,1)) to the 7-cell
lattice, find the containing cell, gather the 16 corner values from
mesh_pred[b] (4096 values), and blend with multilinear weights.

HW constraint (measured): indirect DMA gather consumes ONE index per
partition and streams the dest free-width contiguously from it (multi-
index offset APs abort at runtime, with or without bounds_check).  So
rows are laid out b = n*128 + p (host pre-permutes coordinates into
(p,n) order; output is permuted back) and each of the 32 gathers
fetches, per partition, the 586-float span that covers all 16 cell
corners of one row.  The 32 gathers round-robin over 4 SWDGE queues.
Corner extraction is a fixed multi-dim strided view of the gathered
span; the weighted blend runs group-wise (8 rows-tiles per group) so
it overlaps the remaining gathers.
"""

import numpy as np

import concourse.bass as bass
import concourse.bacc as bacc
import concourse.mybir as mybir
from concourse import bass_utils
from concourse.tile import TileContext

P = 128          # partitions
I = 32           # row-tiles (gathers) per core
GB = 8           # row-tiles per blend group
BC = P * I       # 4096 rows per core
VOL = 4096       # 8^4 lattice values per row
ND = 4
NCORES = 8
MESH = 8
SPANW = 640      # padded per-row gather width (586 used)
SPAN = 586       # 585 max corner offset + 1
NQ = 4           # SWDGE queues
F32 = mybir.dt.float32
I32 = mybir.dt.int32
OP = mybir.AluOpType


def _build():
    nc = bacc.Bacc("TRN2", target_bir_lowering=False, debug=False,
                   num_swdge_queues=NQ)
    # coordinates arrive host-permuted: device row p*I+n = original row n*P+p
    coords = nc.dram_tensor("coordinates", [BC, ND], F32, kind="ExternalInput")
    mesh = nc.dram_tensor("mesh_pred", [BC, VOL], F32, kind="ExternalInput")
    out = nc.dram_tensor("out", [BC], F32, kind="ExternalOutput")

    mesh_2d = mesh[:]
    coords_t = coords[:].rearrange("(p n) d -> p (n d)", p=P)
    out_t = out[:].rearrange("(p n) -> p n", p=P)  # host permutes back

    with TileContext(nc) as tc:
        with tc.tile_pool(name="pool", bufs=1) as pool:
            ct = pool.tile([P, I * ND], F32, tag="ct")
            nc.sync.dma_start(out=ct[:], in_=coords_t)

            # flat row base for original row n*P+p: (n*P+p)*VOL
            # iota pattern steps are int16-limited, so generate n*P+p and
            # shift left by log2(VOL) on DVE (also absorbs the Pool sem)
            tbl = pool.tile([P, I], I32, tag="tbl")
            nc.gpsimd.iota(tbl[:], pattern=[[P, I]], base=0, channel_multiplier=1)
            c = pool.tile([P, I * ND], F32, tag="c")
            nc.vector.tensor_scalar_mul(c[:], ct[:], float(MESH - 1))
            tbl2 = pool.tile([P, I], I32, tag="tbl2")
            nc.vector.tensor_scalar(
                out=tbl2[:], in0=tbl[:], scalar1=12, scalar2=None,
                op0=OP.logical_shift_left,
            )

            # floor(c) via round-trip cast + fixup (any rounding mode: the
            # cast lands on floor or floor+1; is_gt subtracts the overshoot)
            ci_r = pool.tile([P, I * ND], I32, tag="ci_r")
            nc.vector.tensor_copy(out=ci_r[:], in_=c[:])
            ci_f = pool.tile([P, I * ND], F32, tag="ci_f")
            nc.vector.tensor_copy(out=ci_f[:], in_=ci_r[:])
            gt = pool.tile([P, I * ND], F32, tag="gt")
            nc.vector.tensor_tensor(out=gt[:], in0=ci_f[:], in1=c[:], op=OP.is_gt)
            cif = pool.tile([P, I * ND], F32, tag="cif")
            nc.vector.tensor_tensor(out=cif[:], in0=ci_f[:], in1=gt[:], op=OP.subtract)

            frac = pool.tile([P, I * ND], F32, tag="frac")
            nc.vector.tensor_tensor(out=frac[:], in0=c[:], in1=cif[:], op=OP.subtract)

            # lattice idx by Horner: ((d0*8+d1)*8+d2)*8+d3 (exact in f32)
            h0 = pool.tile([P, I], F32, tag="h0")
            nc.vector.tensor_scalar_mul(h0[:], cif[:, 0::ND], 8.0)
            h1 = pool.tile([P, I], F32, tag="h1")
            nc.vector.tensor_tensor(out=h1[:], in0=h0[:], in1=cif[:, 1::ND], op=OP.add)
            h2 = pool.tile([P, I], F32, tag="h2")
            nc.vector.tensor_scalar_mul(h2[:], h1[:], 8.0)
            h3 = pool.tile([P, I], F32, tag="h3")
            nc.vector.tensor_tensor(out=h3[:], in0=h2[:], in1=cif[:, 2::ND], op=OP.add)
            h4 = pool.tile([P, I], F32, tag="h4")
            nc.vector.tensor_scalar_mul(h4[:], h3[:], 8.0)
            idxf = pool.tile([P, I], F32, tag="idxf")
            nc.vector.tensor_tensor(out=idxf[:], in0=h4[:], in1=cif[:, 3::ND], op=OP.add)
            idxi = pool.tile([P, I], I32, tag="idxi")
            nc.vector.tensor_copy(out=idxi[:], in_=idxf[:])
            idx = pool.tile([P, I], I32, tag="idx")
            nc.vector.tensor_tensor(out=idx[:], in0=idxi[:], in1=tbl2[:], op=OP.add)

            # weights: om=1-frac; w01[(g,n)], w23[(j,n)]; W16[(n,k)] k=(a,b,c,d)
            om = pool.tile([P, I * ND], F32, tag="om")
            nc.vector.tensor_scalar(
                out=om[:], in0=frac[:], scalar1=-1.0, scalar2=1.0,
                op0=OP.mult, op1=OP.add,
            )
            w01 = pool.tile([P, 4 * I], F32, tag="w01")
            w23 = pool.tile([P, 4 * I], F32, tag="w23")
            pairs = ((0, 0), (0, 1), (1, 0), (1, 1))
            for g, (a, b) in enumerate(pairs):
                nc.vector.tensor_tensor(
                    out=w23[:, g * I:(g + 1) * I],
                    in0=(frac if a else om)[:, 2::ND],
                    in1=(frac if b else om)[:, 3::ND], op=OP.mult,
                )
            for g, (a, b) in enumerate(pairs):
                nc.vector.tensor_tensor(
                    out=w01[:, g * I:(g + 1) * I],
                    in0=(frac if a else om)[:, 0::ND],
                    in1=(frac if b else om)[:, 1::ND], op=OP.mult,
                )
            W16 = pool.tile([P, I * 16], F32, tag="W16")  # layout (n, k) k fastest
            for k in range(16):
                g, j = k >> 2, k & 3
                nc.vector.tensor_tensor(
                    out=W16[:, k::16],
                    in0=w01[:, g * I:(g + 1) * I],
                    in1=w23[:, j * I:(j + 1) * I], op=OP.mult,
                )

            Gbig = pool.tile([P, I * SPANW], F32, tag="Gbig")
            W16v = W16[:].rearrange("p (n k) -> p n k", k=16)
            acc = pool.tile([P, I], F32, tag="acc")

            # per group of GB row-tiles: issue the gathers (round-robin over
            # SWDGE queues), then blend that group while later groups gather
            for grp in range(0, I, GB):
                for n in range(grp, grp + GB):
                    inst = nc.gpsimd.indirect_dma_start(
                        out=Gbig[:, n * SPANW:n * SPANW + SPAN],
                        out_offset=None,
                        in_=mesh_2d,
                        in_offset=bass.IndirectOffsetOnAxis(
                            ap=idx[:, n:n + 1], axis=1),
                        element_offset=0,
                    )
                    q = n % NQ
                    if q:
                        inst.ins.queue = f"qPoolDynamic{q}"

                # fused blend for this group: per (a,b) corner-pair plane,
                # strided corner view x matching W16 view
                M = []
                for ab in range(4):
                    a, b = ab >> 1, ab & 1
                    goff = grp * SPANW + a * 512 + b * 64
                    gview = Gbig[:]
                    gview = bass.AP(
                        gview.tensor,
                        gview.offset + goff,
                        [gview.ap[0], [SPANW, GB], [8, 2], [1, 2]],
                    )
                    wview = bass.AP(
                        W16v.tensor,
                        W16v.offset + grp * 16 + ab * 4,
                        [W16v.ap[0], [16, GB], [2, 2], [1, 2]],
                    )
                    m = pool.tile([P, GB * 4], F32, tag=f"M{grp}_{ab}")
                    nc.vector.tensor_tensor(
                        out=m[:].rearrange("p (n c d) -> p n c d", c=2, d=2),
                        in0=gview, in1=wview, op=OP.mult,
                    )
                    M.append(m)
                m01 = pool.tile([P, GB * 4], F32, tag=f"m01_{grp}")
                m23 = pool.tile([P, GB * 4], F32, tag=f"m23_{grp}")
                msum = pool.tile([P, GB * 4], F32, tag=f"msum_{grp}")
                nc.vector.tensor_tensor(out=m01[:], in0=M[0][:], in1=M[1][:], op=OP.add)
                nc.vector.tensor_tensor(out=m23[:], in0=M[2][:], in1=M[3][:], op=OP.add)
                nc.vector.tensor_tensor(out=msum[:], in0=m01[:], in1=m23[:], op=OP.add)
                # reduce (c,d): adjacent pairs twice
                t1 = pool.tile([P, GB * 2], F32, tag=f"t1_{grp}")
                nc.vector.tensor_tensor(
                    out=t1[:], in0=msum[:, 0::2], in1=msum[:, 1::2], op=OP.add
                )
                nc.vector.tensor_tensor(
                    out=acc[:, grp:grp + GB], in0=t1[:, 0::2], in1=t1[:, 1::2],
                    op=OP.add,
                )

            nc.sync.dma_start(out=out_t, in_=acc[:])
    nc.compile()
    return nc


_NC = None


def _get_nc():
    global _NC
    if _NC is None:
        _NC = _build()
    return _NC


def kernel(coordinates, mesh_pred, _trace=False, _tmpdir=None):
    coordinates = np.asarray(coordinates, dtype=np.float32)
    mesh_pred = np.asarray(mesh_pred, dtype=np.float32)
    assert coordinates.shape == (NCORES * BC, ND)
    assert mesh_pred.shape == (NCORES * BC, VOL)

    in_maps = []
    for cix in range(NCORES):
        sl = slice(cix * BC, (cix + 1) * BC)
        cs = coordinates[sl]
        # device row p*I+n must hold original row n*P+p
        cs_perm = np.ascontiguousarray(
            cs.reshape(I, P, ND).transpose(1, 0, 2).reshape(BC, ND)
        )
        in_maps.append(
            {
                "coordinates": cs_perm,
                "mesh_pred": np.ascontiguousarray(mesh_pred[sl]),
            }
        )
    res = bass_utils.run_bass_kernel_spmd(
        _get_nc(),
        in_maps,
        core_ids=list(range(NCORES)),
        trace=_trace,
        tmpdir=_tmpdir,
    )
    outs = []
    for r in res.results:
        o = np.asarray(r["out"]).reshape(P, I)  # [p, n]
        outs.append(o.transpose(1, 0).reshape(-1))  # back to b = n*P+p
    out = np.concatenate(outs)
    if _trace:
        return out, res
    return out


# revision 11
# speedup vs baseline: 1.0412x; 1.0412x over previous
"""4D multilinear interpolation (8x8x8x8 lattice) on 8 Trainium2 cores.

For each row b: scale coordinates[b] (4 values in [0,1)) to the 7-cell
lattice, find the containing cell, gather the 16 corner values from
mesh_pred[b] (4096 values), and blend with multilinear weights.

HW constraint (measured): indirect DMA gather consumes ONE index per
partition and streams the dest free-width contiguously from it (multi-
index offset APs abort at runtime, with or without bounds_check).  So
rows are laid out b = n*128 + p (host pre-permutes coordinates into
(p,n) order; output is permuted back) and each of the 32 gathers
fetches, per partition, the 586-float span that covers all 16 cell
corners of one row.  The 32 gathers round-robin over 4 SWDGE queues.
Corner extraction is a fixed multi-dim strided view of the gathered
span; the weighted blend runs group-wise (8 rows-tiles per group) so
it overlaps the remaining gathers.
"""

import numpy as np

import concourse.bass as bass
import concourse.bacc as bacc
import concourse.mybir as mybir
from concourse import bass_utils
from concourse.tile import TileContext

P = 128          # partitions
I = 32           # row-tiles (gathers) per core
GB = 8           # row-tiles per blend group
BC = P * I       # 4096 rows per core
VOL = 4096       # 8^4 lattice values per row
ND = 4
NCORES = 8
MESH = 8
SPANW = 640      # padded per-row gather width (586 used)
SPAN = 586       # 585 max corner offset + 1
NQ = 4           # SWDGE queues
F32 = mybir.dt.float32
I32 = mybir.dt.int32
OP = mybir.AluOpType


def _build():
    nc = bacc.Bacc("TRN2", target_bir_lowering=False, debug=False,
                   dynamic_dma_scratch_size=65536)
    # coordinates arrive host-permuted: device row p*I+n = original row n*P+p
    coords = nc.dram_tensor("coordinates", [BC, ND], F32, kind="ExternalInput")
    mesh = nc.dram_tensor("mesh_pred", [BC, VOL], F32, kind="ExternalInput")
    out = nc.dram_tensor("out", [BC], F32, kind="ExternalOutput")

    mesh_2d = mesh[:]
    coords_t = coords[:].rearrange("(p n) d -> p (n d)", p=P)
    out_t = out[:].rearrange("(p n) -> p n", p=P)  # host permutes back

    with TileContext(nc) as tc:
        with tc.tile_pool(name="pool", bufs=1) as pool:
            ct = pool.tile([P, I * ND], F32, tag="ct")
            nc.sync.dma_start(out=ct[:], in_=coords_t)

            # flat row base for original row n*P+p: (n*P+p)*VOL
            # iota pattern steps are int16-limited, so generate n*P+p and
            # shift left by log2(VOL) on DVE (also absorbs the Pool sem)
            tbl = pool.tile([P, I], I32, tag="tbl")
            nc.gpsimd.iota(tbl[:], pattern=[[P, I]], base=0, channel_multiplier=1)
            c = pool.tile([P, I * ND], F32, tag="c")
            nc.vector.tensor_scalar_mul(c[:], ct[:], float(MESH - 1))
            tbl2 = pool.tile([P, I], I32, tag="tbl2")
            nc.vector.tensor_scalar(
                out=tbl2[:], in0=tbl[:], scalar1=12, scalar2=None,
                op0=OP.logical_shift_left,
            )

            # floor(c) via round-trip cast + fixup (any rounding mode: the
            # cast lands on floor or floor+1; is_gt subtracts the overshoot)
            ci_r = pool.tile([P, I * ND], I32, tag="ci_r")
            nc.vector.tensor_copy(out=ci_r[:], in_=c[:])
            ci_f = pool.tile([P, I * ND], F32, tag="ci_f")
            nc.vector.tensor_copy(out=ci_f[:], in_=ci_r[:])
            gt = pool.tile([P, I * ND], F32, tag="gt")
            nc.vector.tensor_tensor(out=gt[:], in0=ci_f[:], in1=c[:], op=OP.is_gt)
            cif = pool.tile([P, I * ND], F32, tag="cif")
            nc.vector.tensor_tensor(out=cif[:], in0=ci_f[:], in1=gt[:], op=OP.subtract)

            frac = pool.tile([P, I * ND], F32, tag="frac")
            nc.vector.tensor_tensor(out=frac[:], in0=c[:], in1=cif[:], op=OP.subtract)

            # lattice idx by Horner: ((d0*8+d1)*8+d2)*8+d3 (exact in f32)
            h0 = pool.tile([P, I], F32, tag="h0")
            nc.vector.tensor_scalar_mul(h0[:], cif[:, 0::ND], 8.0)
            h1 = pool.tile([P, I], F32, tag="h1")
            nc.vector.tensor_tensor(out=h1[:], in0=h0[:], in1=cif[:, 1::ND], op=OP.add)
            h2 = pool.tile([P, I], F32, tag="h2")
            nc.vector.tensor_scalar_mul(h2[:], h1[:], 8.0)
            h3 = pool.tile([P, I], F32, tag="h3")
            nc.vector.tensor_tensor(out=h3[:], in0=h2[:], in1=cif[:, 2::ND], op=OP.add)
            h4 = pool.tile([P, I], F32, tag="h4")
            nc.vector.tensor_scalar_mul(h4[:], h3[:], 8.0)
            idxf = pool.tile([P, I], F32, tag="idxf")
            nc.vector.tensor_tensor(out=idxf[:], in0=h4[:], in1=cif[:, 3::ND], op=OP.add)
            idxi = pool.tile([P, I], I32, tag="idxi")
            nc.vector.tensor_copy(out=idxi[:], in_=idxf[:])
            idx = pool.tile([P, I], I32, tag="idx")
            nc.vector.tensor_tensor(out=idx[:], in0=idxi[:], in1=tbl2[:], op=OP.add)

            # weights: om=1-frac; w01[(g,n)], w23[(j,n)]; W16[(n,k)] k=(a,b,c,d)
            om = pool.tile([P, I * ND], F32, tag="om")
            nc.vector.tensor_scalar(
                out=om[:], in0=frac[:], scalar1=-1.0, scalar2=1.0,
                op0=OP.mult, op1=OP.add,
            )
            w01 = pool.tile([P, 4 * I], F32, tag="w01")
            w23 = pool.tile([P, 4 * I], F32, tag="w23")
            pairs = ((0, 0), (0, 1), (1, 0), (1, 1))
            for g, (a, b) in enumerate(pairs):
                nc.vector.tensor_tensor(
                    out=w23[:, g * I:(g + 1) * I],
                    in0=(frac if a else om)[:, 2::ND],
                    in1=(frac if b else om)[:, 3::ND], op=OP.mult,
                )
            for g, (a, b) in enumerate(pairs):
                nc.vector.tensor_tensor(
                    out=w01[:, g * I:(g + 1) * I],
                    in0=(frac if a else om)[:, 0::ND],
                    in1=(frac if b else om)[:, 1::ND], op=OP.mult,
                )
            W16 = pool.tile([P, I * 16], F32, tag="W16")  # layout (n, k) k fastest
            for k in range(16):
                g, j = k >> 2, k & 3
                nc.vector.tensor_tensor(
                    out=W16[:, k::16],
                    in0=w01[:, g * I:(g + 1) * I],
                    in1=w23[:, j * I:(j + 1) * I], op=OP.mult,
                )

            Gbig = pool.tile([P, I * SPANW], F32, tag="Gbig")
            W16v = W16[:].rearrange("p (n k) -> p n k", k=16)
            acc = pool.tile([P, I], F32, tag="acc")

            # per group of GB row-tiles: issue the gathers (round-robin over
            # SWDGE queues), then blend that group while later groups gather
            for grp in range(0, I, GB):
                for n in range(grp, grp + GB):
                    nc.gpsimd.indirect_dma_start(
                        out=Gbig[:, n * SPANW:n * SPANW + SPAN],
                        out_offset=None,
                        in_=mesh_2d,
                        in_offset=bass.IndirectOffsetOnAxis(
                            ap=idx[:, n:n + 1], axis=1),
                        element_offset=0,
                    )

                # fused blend for this group: per (a,b) corner-pair plane,
                # strided corner view x matching W16 view
                M = []
                for ab in range(4):
                    a, b = ab >> 1, ab & 1
                    goff = grp * SPANW + a * 512 + b * 64
                    gview = Gbig[:]
                    gview = bass.AP(
                        gview.tensor,
                        gview.offset + goff,
                        [gview.ap[0], [SPANW, GB], [8, 2], [1, 2]],
                    )
                    wview = bass.AP(
                        W16v.tensor,
                        W16v.offset + grp * 16 + ab * 4,
                        [W16v.ap[0], [16, GB], [2, 2], [1, 2]],
                    )
                    m = pool.tile([P, GB * 4], F32, tag=f"M{grp}_{ab}")
                    nc.vector.tensor_tensor(
                        out=m[:].rearrange("p (n c d) -> p n c d", c=2, d=2),
                        in0=gview, in1=wview, op=OP.mult,
                    )
                    M.append(m)
                m01 = pool.tile([P, GB * 4], F32, tag=f"m01_{grp}")
                m23 = pool.tile([P, GB * 4], F32, tag=f"m23_{grp}")
                msum = pool.tile([P, GB * 4], F32, tag=f"msum_{grp}")
                nc.vector.tensor_tensor(out=m01[:], in0=M[0][:], in1=M[1][:], op=OP.add)
                nc.vector.tensor_tensor(out=m23[:], in0=M[2][:], in1=M[3][:], op=OP.add)
                nc.vector.tensor_tensor(out=msum[:], in0=m01[:], in1=m23[:], op=OP.add)
                # reduce (c,d): adjacent pairs twice
                t1 = pool.tile([P, GB * 2], F32, tag=f"t1_{grp}")
                nc.vector.tensor_tensor(
                    out=t1[:], in0=msum[:, 0::2], in1=msum[:, 1::2], op=OP.add
                )
                nc.vector.tensor_tensor(
                    out=acc[:, grp:grp + GB], in0=t1[:, 0::2], in1=t1[:, 1::2],
                    op=OP.add,
                )

            nc.sync.dma_start(out=out_t, in_=acc[:])
    nc.compile()
    return nc


_NC = None


def _get_nc():
    global _NC
    if _NC is None:
        _NC = _build()
    return _NC


def kernel(coordinates, mesh_pred, _trace=False, _tmpdir=None):
    coordinates = np.asarray(coordinates, dtype=np.float32)
    mesh_pred = np.asarray(mesh_pred, dtype=np.float32)
    assert coordinates.shape == (NCORES * BC, ND)
    assert mesh_pred.shape == (NCORES * BC, VOL)

    in_maps = []
    for cix in range(NCORES):
        sl = slice(cix * BC, (cix + 1) * BC)
        cs = coordinates[sl]
        # device row p*I+n must hold original row n*P+p
        cs_perm = np.ascontiguousarray(
            cs.reshape(I, P, ND).transpose(1, 0, 2).reshape(BC, ND)
        )
        in_maps.append(
            {
                "coordinates": cs_perm,
                "mesh_pred": np.ascontiguousarray(mesh_pred[sl]),
            }
        )
    res = bass_utils.run_bass_kernel_spmd(
        _get_nc(),
        in_maps,
        core_ids=list(range(NCORES)),
        trace=_trace,
        tmpdir=_tmpdir,
    )
    outs = []
    for r in res.results:
        o = np.asarray(r["out"]).reshape(P, I)  # [p, n]
        outs.append(o.transpose(1, 0).reshape(-1))  # back to b = n*P+p
    out = np.concatenate(outs)
    if _trace:
        return out, res
    return out
